# revision 27
# baseline (speedup 1.0000x reference)
"""NeuroSAT message-passing GNN on 8 TRN2 NeuronCores (Bass/Tile).

Sharding: clause dim sharded 8-way (2048 clauses/core); literal dim
permuted so core i owns problem i's 500 vars (+12 pads) as 1024 lit rows
(512 pos + 512 neg).  Both message GEMMs are full-K with local output:
  GEMM1: LC[local 2048 clauses] = sum over ALL 8192 lits
  GEMM2: CL[local 1024 lits] = sum over ALL 16384 clauses
L_pre / C_pre are AllGathered in fp8 (split in halves, launched as soon
as each half is staged), packed k-tile-major so every DMA is contiguous.
No ReduceScatter; LSTM inputs come straight from PSUM.  M (counts) is
exact in fp8e4m3; B1 is mostly SBUF-resident (48/64 k-tiles) and B2
partly (32/128), the rest streams from HBM each round through a shared
ring, consumed stream-first so the resident tail keeps the PE busy while
the next stream window opens.
"""

import numpy as np
import ml_dtypes

import concourse.bass as bass
import concourse.bacc as bacc
import concourse.mybir as mybir
import concourse.tile as tile
from concourse import bass_utils

F32 = mybir.dt.float32
BF16 = mybir.dt.bfloat16
FP8 = mybir.dt.float8e4
AF = mybir.ActivationFunctionType
DR = mybir.MatmulPerfMode.DoubleRow

N_CORES = 8
DIM = 128
N_ROUNDS = 16
N_VARS = 4000
VPC = 500            # real vars per core (= vars per problem)
VPAD = 512           # padded vars per core
LL = 2 * VPAD        # 1024 lit rows per core
LPAD = N_CORES * LL  # 8192
CC = 2048            # clauses per core
CPAD = N_CORES * CC  # 16384
KL = LPAD // 128     # 64 k-tiles over lits (GEMM1 contraction)
KC2 = CPAD // 128    # 128 k-tiles over clauses (GEMM2 contraction)
KCL = CC // 128      # 16 local clause k-tiles

B1_RES = 48          # B1 k-tiles resident in SBUF (rest streamed)
B1_STR = KL - B1_RES
B2_RES = 28          # B2 k-tiles resident (tail of consumption order)
B2_STR = KC2 - B2_RES
CH1 = 2              # B1 k-tiles per stream chunk ([128, 4096] = 512KB)
CH2 = 4              # B2 k-tiles per stream chunk ([128, 4096] = 512KB)

N_WARM_G1 = 24       # dummy MMs covering AG_L0 + readback
N_WARM_G2 = 32       # dummy MMs covering AG_C0 + readback
N_FILL = 4           # dummy MMs per streamed B2 chunk (HAM duty)
N_FILL1 = 2          # dummy MMs per streamed B1 chunk

nbf = ml_dtypes.bfloat16
nf8 = ml_dtypes.float8_e4m3

# k-tile consumption orders (host pack order must match):
# AG halves deliver [all cores' first-half tiles][all cores' second-half].
ORD1 = [c8 * 8 + t for half in (0, 1) for c8 in range(N_CORES)
        for t in range(4 * half, 4 * half + 4)]
ORD2 = [c8 * 16 + t for half in (0, 1) for c8 in range(N_CORES)
        for t in range(8 * half, 8 * half + 8)]

_CACHE = {}


def _build():
    """Build + compile the SPMD program once (shape-only, no input values)."""
    if "nc" in _CACHE:
        return _CACHE["nc"]

    nc = bacc.Bacc("TRN2", target_bir_lowering=False, debug=False,
                   num_devices=N_CORES)

    def din(name, shape, dt):
        return nc.dram_tensor(name, shape, dt, kind="ExternalInput")

    b1_res = din("b1_res", [DIM, B1_RES * CC], FP8)
    b1_str = din("b1_str", [DIM, B1_STR * CC], FP8)
    b2_res = din("b2_res", [DIM, B2_RES * LL], FP8)
    b2_str = din("b2_str", [DIM, B2_STR * LL], FP8)
    lh0t = din("lh0t", [DIM, LL], BF16)
    ch0t = din("ch0t", [DIM, CC], BF16)
    id128 = din("id128", [DIM, DIM], BF16)

    w = {}
    for p in ("lmsg", "cmsg", "lvote"):
        for i in (1, 2, 3):
            shp = [DIM, 1] if (p == "lvote" and i == 3) else [DIM, DIM]
            w[f"{p}_w{i}t"] = din(f"{p}_w{i}t", shp, BF16)
            bshp = [1, 1] if (p == "lvote" and i == 3) else [DIM, 1]
            w[f"{p}_b{i}"] = din(f"{p}_b{i}", bshp, F32)
    w["cu_wiht"] = din("cu_wiht", [DIM, 4 * DIM], BF16)
    w["cu_whht"] = din("cu_whht", [DIM, 4 * DIM], BF16)
    w["lu_wiht_cl"] = din("lu_wiht_cl", [DIM, 4 * DIM], BF16)
    w["lu_wiht_fl"] = din("lu_wiht_fl", [DIM, 4 * DIM], BF16)
    w["lu_whht"] = din("lu_whht", [DIM, 4 * DIM], BF16)
    cu_bias_d = din("cu_bias", [4, DIM], F32)
    lu_bias_d = din("lu_bias", [4, DIM], F32)

    vote_out = nc.dram_tensor("vote", [1, LL], F32, kind="ExternalOutput")

    with tile.TileContext(nc) as tc, \
         tc.tile_pool(name="const", bufs=1) as const, \
         tc.tile_pool(name="sb", bufs=2) as sb, \
         tc.tile_pool(name="sb1", bufs=1) as sb1, \
         tc.tile_pool(name="sbs", bufs=2) as sbs, \
         tc.tile_pool(name="ps", bufs=6, space="PSUM") as ps, \
         tc.tile_pool(name="pstr", bufs=2, space="PSUM") as pstr, \
         tc.tile_pool(name="dram", bufs=2, space="DRAM") as dram:

        # ---- load constants/weights into SBUF
        cw = {}
        for k in w:
            t = const.tile(list(w[k].shape), w[k].dtype, tag=f"cw_{k}")
            nc.sync.dma_start(t[:], w[k].ap())
            cw[k] = t
        for k, dte in (("cu_bias", cu_bias_d), ("lu_bias", lu_bias_d)):
            t = const.tile([DIM, 4], F32, tag=f"cw_{k}")
            nc.sync.dma_start(t[:], dte.ap().rearrange("g p -> p g"))
            cw[k] = t
        idt = const.tile([DIM, DIM], BF16, tag="idt")
        nc.sync.dma_start(idt[:], id128.ap())
        zst = const.tile([DIM, DIM], BF16, tag="zst")
        nc.vector.memset(zst[:], 0.0)

        # ---- resident operand blocks, loaded once
        b1r = const.tile([DIM, B1_RES * CC], FP8, tag="b1r")
        for q in range(4):
            qc = B1_RES * CC // 4
            nc.sync.dma_start(b1r[:, q * qc:(q + 1) * qc],
                              b1_res.ap()[:, q * qc:(q + 1) * qc])
        b2r = const.tile([DIM, B2_RES * LL], FP8, tag="b2r")
        nc.sync.dma_start(b2r[:], b2_res.ap())

        # ---- persistent state (feature-major)
        lht = const.tile([DIM, LL], BF16, tag="lht")
        lct = const.tile([DIM, LL], F32, tag="lct")
        cht = const.tile([DIM, CC], BF16, tag="cht")
        cct = const.tile([DIM, CC], BF16, tag="cct")
        nc.sync.dma_start(lht[:], lh0t.ap())
        nc.sync.dma_start(cht[:], ch0t.ap())
        nc.vector.memset(lct[:], 0.0)
        nc.vector.memset(cct[:], 0.0)

        # stages for AG inputs (k-tile-major fp8) and gathered operands
        lpre_stage = const.tile([DIM, 8 * DIM], FP8, tag="lpre_stage")
        cpre_stage = const.tile([DIM, KCL * DIM], FP8, tag="cpre_stage")
        lpre_all = const.tile([DIM, KL * DIM], FP8, tag="lpre_all")
        cpre_all = const.tile([DIM, KC2 * DIM], FP8, tag="cpre_all")

        def transpose_to_stage(src, src0, stage, t0, nt, pfx):
            """PE-transpose nt 128-col tiles of src (from col src0) into
            stage k-tile slots starting at t0, casting to fp8."""
            for t in range(nt):
                sl = slice(src0 + t * DIM, src0 + (t + 1) * DIM)
                dsl = slice((t0 + t) * DIM, (t0 + t + 1) * DIM)
                pt = pstr.tile([DIM, DIM], BF16, tag="pstr", name=f"{pfx}_tr")
                nc.tensor.transpose(pt[:], src[:, sl], idt[:])
                nc.vector.tensor_copy(stage[:, dsl], pt[:])

        def mlp_to_stage(x, pfx, sl, stage, t0):
            """3-layer MLP on columns sl of x, chunk-wise (512 cols), each
            chunk transposed into stage fp8 k-tiles."""
            n = sl.stop - sl.start
            for rc in range(n // 512):
                cur = None
                for li in (1, 2, 3):
                    wt = cw[f"{pfx}_w{li}t"]
                    bt = cw[f"{pfx}_b{li}"]
                    pt = ps.tile([DIM, 512], F32, tag="ps", name="mlp_ps")
                    src = x[:, sl.start + rc * 512:sl.start + rc * 512 + 512] \
                        if li == 1 else cur[:]
                    nc.tensor.matmul(pt[:], wt[:], src, start=True, stop=True)
                    func = AF.Relu if li < 3 else AF.Identity
                    o = sb.tile([DIM, 512], BF16, tag=f"mlp_h{li}",
                                name=f"{pfx}_h{li}")
                    nc.scalar.activation(o[:], pt[:], func, bias=bt[:, 0:1])
                    cur = o
                transpose_to_stage(cur, 0, stage, t0 + rc * 4, 4, pfx)

        def lstm_elementwise(gps, bias, c_st, h_st, rc0, n):
            """gps: 4 psum tiles [128, n] (i,f,g,o); updates states [:, rc0:rc0+n]."""
            sl = slice(rc0, rc0 + n)
            sig_i = sb1.tile([DIM, n], BF16, tag="lw_si", name="sig_i")
            sig_f = sb1.tile([DIM, n], BF16, tag="lw_sf", name="sig_f")
            tng = sb1.tile([DIM, n], BF16, tag="lw_tg", name="tng")
            sig_o = sb1.tile([DIM, n], BF16, tag="lw_so", name="sig_o")
            nc.scalar.activation(sig_i[:], gps[0][:], AF.Sigmoid, bias=bias[:, 0:1])
            nc.scalar.activation(sig_f[:], gps[1][:], AF.Sigmoid, bias=bias[:, 1:2])
            nc.scalar.activation(tng[:], gps[2][:], AF.Tanh, bias=bias[:, 2:3])
            nc.scalar.activation(sig_o[:], gps[3][:], AF.Sigmoid, bias=bias[:, 3:4])
            t1 = sb1.tile([DIM, n], BF16, tag="lw_t1", name="t1")
            nc.vector.tensor_mul(t1[:], sig_f[:], c_st[:, sl])
            t2 = sb1.tile([DIM, n], BF16, tag="lw_t2", name="t2")
            nc.vector.tensor_mul(t2[:], sig_i[:], tng[:])
            nc.vector.tensor_add(c_st[:, sl], t1[:], t2[:])
            tnc = sb1.tile([DIM, n], BF16, tag="lw_tc", name="tnc")
            nc.scalar.activation(tnc[:], c_st[:, sl], AF.Tanh)
            nc.vector.tensor_mul(h_st[:, sl], sig_o[:], tnc[:])

        rg = [list(range(N_CORES))]

        def collective(kind, op, cin, cout):
            nc.gpsimd.collective_compute(kind, op, replica_groups=rg,
                                         ins=[cin.opt()], outs=[cout.opt()])

        def ag_launch(stage, c0, n, htag, r):
            """DMA stage[:, c0:c0+n] -> DRAM, AllGather, return out tile."""
            cin = dram.tile([DIM, n], FP8, tag=f"ag_in_{htag}",
                            name=f"ag_in_{htag}_{r}")
            nc.sync.dma_start(cin[:], stage[:, c0:c0 + n])
            cout = dram.tile([N_CORES * DIM, n], FP8, tag=f"ag_out_{htag}",
                             name=f"ag_out_{htag}_{r}")
            collective("AllGather", mybir.AluOpType.bypass, cin, cout)
            return cout

        def ag_read(cout, dst, d0, n):
            """Read AG output blocks into dst cols d0.. (one merged DMA)."""
            src = cout[:, :].rearrange("(c p) n -> p c n", p=DIM)
            d3 = dst[:, d0:d0 + N_CORES * n].rearrange("p (c n) -> p c n", n=n)
            nc.sync.dma_start(d3, src)

        def warm(n_warm, tgt, nt):
            for wi in range(n_warm):
                nc.tensor.matmul(tgt[wi % nt][:], zst[:], lht[:, 0:512],
                                 start=(wi < nt), stop=False)

        def gemm1(n_warm):
            """LC.T [128 d, 2048 c]: resident 48 k-tiles then streamed 16."""
            scratch = ps.tile([DIM, 512], F32, tag="ps", name="g1_scratch")
            g1 = [ps.tile([DIM, 512], F32, tag="ps", name=f"g1_{i}")
                  for i in range(4)]
            warm(n_warm, g1, 4)
            fs = [False]

            def fill(k):
                for _ in range(k):
                    nc.tensor.matmul(scratch[:], zst[:], lht[:, 0:512],
                                     start=not fs[0], stop=False)
                    fs[0] = True

            b1v = b1r[:].rearrange("p (t c) -> p t c", c=CC)
            for g in range(0, KL, 2):
                if g < B1_RES:
                    mv = b1v[:, g:g + 2, :]
                else:
                    t = sbs.tile([DIM, CH1 * CC], FP8, tag="ring", name="b1ch")
                    c0 = (g - B1_RES) * CC
                    nc.sync.dma_start(t[:], b1_str.ap()[:, c0:c0 + CH1 * CC])
                    mv = t[:].rearrange("p (t c) -> p t c", c=CC)[:, 0:2, :]
                lk = lpre_all[:, g * DIM:(g + 2) * DIM].rearrange(
                    "p (j d) -> p j d", j=2)
                for cc in range(4):
                    nc.tensor.matmul(
                        g1[cc][:], lk, mv[:, :, cc * 512:(cc + 1) * 512],
                        start=(g == 0 and n_warm == 0), stop=(g == KL - 2),
                        perf_mode=DR)
                if g >= B1_RES:
                    fill(N_FILL1)
            return g1

        def gemm2(pre, n_warm):
            """CL.T [128 d, 1024 l]: 96 streamed k-tiles then 32 resident."""
            scratch = ps.tile([DIM, 512], F32, tag="ps", name="g2_scratch")
            g2 = [ps.tile([DIM, 512], F32, tag="ps", name=f"g2_{i}")
                  for i in range(2)]
            warm(n_warm, g2, 2)
            fs = [False]

            def fill(k):
                for _ in range(k):
                    nc.tensor.matmul(scratch[:], zst[:], lht[:, 0:512],
                                     start=not fs[0], stop=False)
                    fs[0] = True

            def mm_pair(pos, mv2):
                """One DR group: consumption pos (pos, pos+1) k-tiles."""
                ck = cpre_all[:, pos * DIM:(pos + 2) * DIM].rearrange(
                    "p (j d) -> p j d", j=2)
                for h in range(2):
                    nc.tensor.matmul(
                        g2[h][:], ck, mv2[:, :, h * 512:(h + 1) * 512],
                        start=(pos == 0 and n_warm == 0),
                        stop=(pos == KC2 - 2), perf_mode=DR)

            for ch in range(B2_STR // CH2):
                if ch < len(pre):
                    b2t = pre[ch]
                else:
                    b2t = sbs.tile([DIM, CH2 * LL], FP8, tag="ring",
                                   name="b2ch")
                    c0 = ch * CH2 * LL
                    nc.sync.dma_start(b2t[:],
                                      b2_str.ap()[:, c0:c0 + CH2 * LL])
                b2c = b2t[:].rearrange("p (t l) -> p t l", l=LL)
                for kk in range(0, CH2, 2):
                    mm_pair(ch * CH2 + kk, b2c[:, kk:kk + 2, :])
                fill(N_FILL)
            b2rv = b2r[:].rearrange("p (t l) -> p t l", l=LL)
            for t in range(0, B2_RES, 2):
                mm_pair(B2_STR + t, b2rv[:, t:t + 2, :])
            return g2

        def gemm2_prefetch():
            """First ring-depth B2 chunk DMAs, issued before AG_C."""
            pre = []
            for ch in range(2):
                b2t = sbs.tile([DIM, CH2 * LL], FP8, tag="ring", name="b2pre")
                c0 = ch * CH2 * LL
                nc.sync.dma_start(b2t[:], b2_str.ap()[:, c0:c0 + CH2 * LL])
                pre.append(b2t)
            return pre

        def c_phase(g1, r):
            """C-LSTM + C_pre MLP -> cpre_stage; AG_C halves launched as
            soon as each half of cpre_stage is staged."""
            outs = []
            lc_full = sb1.tile([DIM, CC], BF16, tag="lc_full", name="lc_full")
            for rc in range(4):
                nc.vector.tensor_copy(lc_full[:, rc * 512:(rc + 1) * 512],
                                      g1[rc][:])
            for rc in range(4):
                sl = slice(rc * 512, (rc + 1) * 512)
                gps = [ps.tile([DIM, 512], F32, tag="ps", name=f"cg{i}")
                       for i in range(4)]
                for g in range(4):
                    gsl = slice(g * DIM, (g + 1) * DIM)
                    nc.tensor.matmul(gps[g][:], cw["cu_wiht"][:, gsl],
                                     lc_full[:, sl], start=True, stop=False)
                    nc.tensor.matmul(gps[g][:], cw["cu_whht"][:, gsl],
                                     cht[:, sl], start=False, stop=True)
                lstm_elementwise(gps, cw["cu_bias"], cct, cht, rc * 512, 512)
                mlp_to_stage(cht, "cmsg", sl, cpre_stage, rc * 4)
                if rc == 1:
                    outs.append(ag_launch(cpre_stage, 0, 8 * DIM, "c0", r))
                if rc == 3:
                    outs.append(ag_launch(cpre_stage, 8 * DIM, 8 * DIM,
                                          "c1", r))
            return outs

        def l_phase(g2, r):
            """L-LSTM both halves + L_pre MLP; AG_L halves launched per half."""
            outs = []
            lh_flip = sb1.tile([DIM, LL], BF16, tag="lh_flip", name="lh_flip")
            nc.vector.tensor_copy(lh_flip[:], lht[:])
            clt = sb1.tile([DIM, LL], BF16, tag="clt", name="clt")
            for h in range(2):
                nc.vector.tensor_copy(clt[:, h * 512:(h + 1) * 512], g2[h][:])
            for h in range(2):
                sl = slice(h * 512, (h + 1) * 512)
                fsl = slice((1 - h) * 512, (2 - h) * 512)
                gps = [ps.tile([DIM, 512], F32, tag="ps", name=f"lg{h}_{i}")
                       for i in range(4)]
                for g in range(4):
                    gsl = slice(g * DIM, (g + 1) * DIM)
                    nc.tensor.matmul(gps[g][:], cw["lu_wiht_cl"][:, gsl],
                                     clt[:, sl], start=True, stop=False)
                    nc.tensor.matmul(gps[g][:], cw["lu_wiht_fl"][:, gsl],
                                     lh_flip[:, fsl], start=False, stop=False)
                    nc.tensor.matmul(gps[g][:], cw["lu_whht"][:, gsl],
                                     lh_flip[:, sl], start=False, stop=True)
                lstm_elementwise(gps, cw["lu_bias"], lct, lht, h * 512, 512)
                mlp_to_stage(lht, "lmsg", sl, lpre_stage, h * 4)
                outs.append(ag_launch(lpre_stage, h * 512, 4 * DIM,
                                      f"l{h}", r))
            return outs

        # ====== round 0 head: L_pre from Lh0 -> lpre_stage + AG launches
        agl = []
        for h in range(2):
            mlp_to_stage(lht, "lmsg", slice(h * 512, (h + 1) * 512),
                         lpre_stage, h * 4)
            agl.append(ag_launch(lpre_stage, h * 512, 4 * DIM, f"l{h}", -1))

        for r in range(N_ROUNDS):
            ag_read(agl[0], lpre_all, 0, 4 * DIM)
            ag_read(agl[1], lpre_all, 32 * DIM, 4 * DIM)
            g1 = gemm1(N_WARM_G1)
            # ring-allocated after gemm1's chunks so allocation order matches
            # consumption order (b1 chunks -> b2 prefetch -> b2 inline)
            pre = gemm2_prefetch()
            agc = c_phase(g1, r)
            ag_read(agc[0], cpre_all, 0, 8 * DIM)
            ag_read(agc[1], cpre_all, 64 * DIM, 8 * DIM)
            g2 = gemm2(pre, N_WARM_G2)
            agl = l_phase(g2, r)

        # ---- vote MLP on final Lh -> [1, 1024] f32
        for h in range(2):
            cur = None
            for li in (1, 2, 3):
                wt = cw[f"lvote_w{li}t"]
                bt = cw[f"lvote_b{li}"]
                m = wt.shape[1]
                pt = ps.tile([m, 512], F32, tag="ps", name="vote_ps")
                src = lht[:, h * 512:(h + 1) * 512] if li == 1 else cur[:]
                nc.tensor.matmul(pt[:], wt[:], src, start=True, stop=True)
                func = AF.Relu if li < 3 else AF.Identity
                if li < 3:
                    o = sb.tile([DIM, 512], BF16, tag=f"mlp_h{li}",
                                name=f"vote_h{li}")
                else:
                    o = sb.tile([1, 512], F32, tag="vote_o", name="vote_o")
                nc.scalar.activation(o[:], pt[:], func, bias=bt[:, 0:1])
                cur = o
            nc.sync.dma_start(vote_out.ap()[:, h * 512:(h + 1) * 512], cur[:])

    nc.compile()
    _CACHE["nc"] = nc
    return nc


def _perm_rows(lits):
    """Map global lit index -> permuted row (core-major, 1024 rows/core)."""
    lits = np.asarray(lits)
    neg = lits >= N_VARS
    v = np.where(neg, lits - N_VARS, lits)
    core = v // VPC
    r = v % VPC
    return core * LL + np.where(neg, VPAD + r, r)


def host_prep(inp):
    f32 = np.float32
    idx = inp["L_unpack_indices"].astype(np.int64)
    rows = _perm_rows(idx[:, 0])
    M = np.zeros((LPAD, CPAD), np.float32)
    np.add.at(M, (rows, idx[:, 1]), 1.0)

    maps = []
    for i in range(N_CORES):
        blk = M[:, i * CC:(i + 1) * CC]           # [8192, 2048]
        b1t = blk.reshape(KL, DIM, CC)[ORD1]      # [64, 128, 2048] in ORD1
        b1p = b1t.transpose(1, 0, 2).reshape(DIM, KL * CC)
        rblk = M[i * LL:(i + 1) * LL, :].T        # [16384, 1024]
        b2t = rblk.reshape(KC2, DIM, LL)[ORD2]    # [128, 128, 1024] in ORD2
        b2p = b2t.transpose(1, 0, 2).reshape(DIM, KC2 * LL)
        maps.append({
            "b1_res": np.ascontiguousarray(b1p[:, :B1_RES * CC]).astype(nf8),
            "b1_str": np.ascontiguousarray(b1p[:, B1_RES * CC:]).astype(nf8),
            "b2_str": np.ascontiguousarray(b2p[:, :B2_STR * LL]).astype(nf8),
            "b2_res": np.ascontiguousarray(b2p[:, B2_STR * LL:]).astype(nf8),
        })

    def bf(x):
        return np.ascontiguousarray(x).astype(nbf)

    l0 = (inp["L_init_w"][:, 0] + inp["L_init_b"]).astype(f32)
    c0 = (inp["C_init_w"][:, 0] + inp["C_init_b"]).astype(f32)
    common = {
        "lh0t": bf(np.repeat(l0[:, None], LL, axis=1)),
        "ch0t": bf(np.repeat(c0[:, None], CC, axis=1)),
        "id128": bf(np.eye(DIM, dtype=f32)),
        "cu_wiht": bf(inp["Cu_wih"].T), "cu_whht": bf(inp["Cu_whh"].T),
        "lu_wiht_cl": bf(inp["Lu_wih"].T[:DIM]),
        "lu_wiht_fl": bf(inp["Lu_wih"].T[DIM:]),
        "lu_whht": bf(inp["Lu_whh"].T),
        "cu_bias": (inp["Cu_bih"] + inp["Cu_bhh"]).astype(f32).reshape(4, DIM),
        "lu_bias": (inp["Lu_bih"] + inp["Lu_bhh"]).astype(f32).reshape(4, DIM),
    }
    for p, P in (("lmsg", "Lmsg"), ("cmsg", "Cmsg"), ("lvote", "Lvote")):
        for i in (1, 2, 3):
            common[f"{p}_w{i}t"] = bf(inp[f"{P}_w{i}"].T)
            bshape = (1, 1) if (p == "lvote" and i == 3) else (DIM, 1)
            common[f"{p}_b{i}"] = inp[f"{P}_b{i}"].astype(f32).reshape(bshape)
    return [dict(common, **maps[i]) for i in range(N_CORES)]


def kernel(**inputs):
    inp = {k: np.asarray(v) for k, v in inputs.items()}
    in_maps = host_prep(inp)
    nc = _build()
    res = bass_utils.run_bass_kernel_spmd(nc, in_maps,
                                          core_ids=list(range(N_CORES)))
    probs = np.zeros(N_CORES, np.float32)
    for i in range(N_CORES):
        v = res.results[i]["vote"][0]            # [1024]
        s = v[:VPC].astype(np.float64).sum() + \
            v[VPAD:VPAD + VPC].astype(np.float64).sum()
        probs[i] = np.float32(s / (2 * VPC))
    return probs
